# revision 6
# baseline (speedup 1.0000x reference)
"""Multi-head attention + output projection on 8 Trainium2 NeuronCores.

Problem (hardcoded): x [3, 2, 4096, 512] fp32 (q/k/v stacked), proj_w [512, 512],
proj_b [512].  reference = softmax(q k^T / sqrt(64)) v, heads=8, then
out @ proj_w.T + proj_b.

Sharding: B*H = 16 (batch, head) pairs over 8 cores -> each core gets one
batch and one adjacent head PAIR (2 heads = 128 feature dims).  The output
projection is tensor-parallel along the contraction dim: each core computes
its partial y = attn_out_pair @ W[:, pair_dims].T; the host sums the 4
partials per batch and adds the bias.

Device kernel (per core).  All matmul operands bf16, PSUM accumulation fp32:
  scores^T[nk, nq] = kT_chunk.T @ qT      (two heads row-tiled in the PE
                                           array: K=64 each at rows 0-63 /
                                           64-127, concurrent)
  P^T = exp(0.125 * scores^T)             split across TWO engines:
      - ScalarE ACTIVATE (exact exp, bf16 out) for 18/32 chunks
      - VectorE Schraudolph bit-trick exp for 14: one tensor_scalar
        i16 = round(s*A + B), bitcast to bf16 (max rel err ~3.5%, washes
        out in the softmax weighted mean)
  acc[d, nq]  += [V | 1].T @ P^T          (K=128; row 64 = softmax denom)
  proj: st_h = acc_h * (1/den_h) broadcast  (GpSimd multiply; reciprocal
        on DVE in a [128,4,2] transposed layout, broadcast via DMA), then
  y[nq, o]    = st_h0^T @ W_h0 + st_h1^T @ W_h1   (accumulated in PSUM),
        copied to SBUF (DVE) and DMA'd out.

Pipeline: 3 score buffers in PSUM (6 banks) + 2 accumulator banks; proj y
tiles borrow score-pool buffers.  PV held back 4 chunks so the exp engines
have ~2.5us of slack per tile; the three engines are balanced at ~20-22us
per 512-query block.
"""

import numpy as np

C, B, N, D, H = 3, 2, 4096, 512, 8
HD = 64          # head dim
NCORES = 8
NQB = 512        # nq block width (PSUM bank)
NBLK = N // NQB  # 8 nq blocks
NCHUNK = N // 128  # 32 nk chunks of 128
PV_HOLD = 4      # chunks of slack between scores and PV consumption

# chunks whose exp runs on the VectorE (Schraudolph); rest on ScalarE.
# Odd chunks from 5: the first chunks all go to ScalarE so the DVE can
# drain the previous block's accumulator staging copies first.
DVE_CHUNKS = frozenset(ck for ck in range(5, NCHUNK, 2))  # 14 of 32

# Schraudolph constants for bf16-target exp(0.125*s):
#   i16 = round(s * A16 + B16); bitcast i16 -> bf16 approximates exp(s/8).
A16 = float(2.0 ** 7 * np.log2(np.e) * 0.125)
B16 = float(16256.0 - 128.0 * 0.05)

_compiled = None


def _build_nc():
    import concourse.bacc as bacc
    import concourse.tile as tile
    from concourse import mybir

    f32 = mybir.dt.float32
    bf16 = mybir.dt.bfloat16
    i16 = mybir.dt.int16
    Exp = mybir.ActivationFunctionType.Exp
    mult = mybir.AluOpType.mult
    add = mybir.AluOpType.add

    nc = bacc.Bacc("TRN2", target_bir_lowering=False, debug=False, num_devices=1)

    qT = nc.dram_tensor("qT", [128, N], bf16, kind="ExternalInput").ap()
    kT = nc.dram_tensor("kT", [128, N], bf16, kind="ExternalInput").ap()
    vI = nc.dram_tensor("vI", [128, NCHUNK, 2, HD + 1], bf16, kind="ExternalInput").ap()
    wT = nc.dram_tensor("wT", [HD, 2, D], bf16, kind="ExternalInput").ap()
    y = nc.dram_tensor("y", [N, D], f32, kind="ExternalOutput").ap()

    with tile.TileContext(nc) as tc:
        with (
            tc.tile_pool(name="const", bufs=1) as const_pool,
            tc.tile_pool(name="pt", bufs=5) as pt_pool,
            tc.tile_pool(name="ep", bufs=2) as ep_pool,
            tc.tile_pool(name="ps_s", bufs=3, space="PSUM") as ps_s,
            tc.tile_pool(name="ps_a", bufs=1, space="PSUM") as ps_a,
            tc.tile_pool(name="dram", bufs=2, space="DRAM") as dram_pool,
        ):
            # resident inputs
            qT_sb = const_pool.tile([128, N], bf16)
            kT_sb = const_pool.tile([128, N], bf16)
            vI_sb = const_pool.tile([128, NCHUNK, 2, HD + 1], bf16)
            # trigger the exp table load while the input DMAs stream in
            warm = ep_pool.tile([128, 2], f32, tag="warm")
            nc.vector.memset(warm[:], 0.0)
            nc.scalar.activation(warm[:, 1:2], warm[:, 0:1], Exp)
            nc.sync.dma_start(kT_sb[:, 0:128], kT[:, 0:128])
            nc.sync.dma_start(qT_sb[:, 0:NQB], qT[:, 0:NQB])
            nc.sync.dma_start(kT_sb[:, 128:512], kT[:, 128:512])
            nc.gpsimd.dma_start(vI_sb[:, 0:4], vI[:, 0:4])
            for ck4 in range(4, NCHUNK, 4):
                sl = slice(ck4 * 128, (ck4 + 4) * 128)
                nc.sync.dma_start(kT_sb[:, sl], kT[:, sl])
                nc.gpsimd.dma_start(vI_sb[:, ck4:ck4 + 4], vI[:, ck4:ck4 + 4])
            for b in range(1, NBLK):
                nc.gpsimd.dma_start(qT_sb[:, b * NQB:(b + 1) * NQB],
                                    qT[:, b * NQB:(b + 1) * NQB])
            wT_sb = const_pool.tile([HD, 2, D], bf16)
            nc.sync.dma_start(wT_sb[:], wT[:])
            # zero weights for the HAM-warming filler matmuls (wiped by the
            # start=True PV below; keeps the PE dense so the clock gate
            # trips to 8/8 before block 0 starts)
            zeros_sb = const_pool.tile([128, NQB], bf16)
            nc.vector.memset(zeros_sb[:], 0.0)

            pending_projs = []
            for blk in range(NBLK):
                q0 = blk * NQB
                a_h0 = ps_a.tile([HD + 1, NQB], f32, tag="a_h0")
                a_h1 = ps_a.tile([HD + 1, NQB], f32, tag="a_h1")
                if blk == 0:
                    for f in range(20):
                        nc.tensor.matmul(
                            (a_h0 if f % 2 == 0 else a_h1)[:],
                            lhsT=zeros_sb[:, 0:HD + 1], rhs=zeros_sb[:],
                            start=False, stop=False)
                # Software pipeline: PV for chunk t runs PV_HOLD chunks
                # behind the scores matmuls, so exp latency jitter on the
                # ScalarE/VectorE never stalls the PE.
                pv_queue = []

                def emit_pv(args):
                    pt_, ck_ = args
                    first = ck_ == 0
                    last = ck_ == NCHUNK - 1
                    nc.tensor.matmul(
                        a_h0[:], lhsT=vI_sb[:, ck_, 0, :],
                        rhs=pt_[:, 0:NQB], start=first, stop=last)
                    nc.tensor.matmul(
                        a_h1[:], lhsT=vI_sb[:, ck_, 1, :],
                        rhs=pt_[:, NQB:2 * NQB], start=first, stop=last)

                for ck in range(NCHUNK):
                    # one [128, 1024] scores tile per chunk: h0 in bank 0,
                    # h1 in bank 1, the two matmuls run as a concurrent
                    # row-tiled pair (K=64 at array rows 0 / 64).
                    s_t = ps_s.tile([128, 2 * NQB], f32, tag="s_t")
                    nc.tensor.matmul(
                        s_t[:, 0:NQB],
                        lhsT=kT_sb[0:HD, ck * 128:(ck + 1) * 128],
                        rhs=qT_sb[0:HD, q0:q0 + NQB],
                        start=True, stop=True)
                    nc.tensor.matmul(
                        s_t[:, NQB:2 * NQB],
                        lhsT=kT_sb[HD:128, ck * 128:(ck + 1) * 128],
                        rhs=qT_sb[HD:128, q0:q0 + NQB],
                        start=True, stop=True)
                    if ck in DVE_CHUNKS:
                        pti = pt_pool.tile([128, 2 * NQB], i16, tag="ptd")
                        nc.vector.tensor_scalar(
                            pti[:], s_t[:], A16, B16, mult, add)
                        pt = pti[:].bitcast(bf16)
                    else:
                        pt = pt_pool.tile([128, 2 * NQB], bf16, tag="pta")
                        nc.scalar.activation(pt[:], s_t[:], Exp, scale=0.125)
                    pv_queue.append((pt, ck))
                    if len(pv_queue) > PV_HOLD:
                        emit_pv(pv_queue.pop(0))
                    # previous block's proj groups, spread out late enough
                    # that the reciprocal-denominator DMA chain is done
                    if ck in (10, 16, 22, 27) and pending_projs:
                        pending_projs.pop(0)()
                for a in pv_queue:
                    emit_pv(a)

                # Stage accumulators to SBUF right away: this is the only
                # thing the next block's PV accumulation waits on.  The den
                # rows live at PSUM partition 64; DVE lanes are partition-
                # aligned so the staging tile keeps them there.
                dtmp = dram_pool.tile([2, NQB], f32, tag="dtmp")
                rden_d = dram_pool.tile([2, NQB], f32, tag="rden_d")
                den_sb = ep_pool.tile([HD + 1, 2, NQB], f32, tag="den_sb")
                nc.vector.tensor_copy(den_sb[HD:HD + 1, 0, :], a_h0[HD:HD + 1, :])
                st0 = ep_pool.tile([HD, NQB], f32, tag="st0")
                nc.vector.tensor_copy(st0[:], a_h0[0:HD, :])
                nc.sync.dma_start(dtmp[0:1, :], den_sb[HD:HD + 1, 0, :])
                nc.vector.tensor_copy(den_sb[HD:HD + 1, 1, :], a_h1[HD:HD + 1, :])
                st1 = ep_pool.tile([HD, NQB], f32, tag="st1")
                nc.vector.tensor_copy(st1[:], a_h1[0:HD, :])
                nc.sync.dma_start(dtmp[1:2, :], den_sb[HD:HD + 1, 1, :])
                # reciprocal in a [128, 4, 2] transposed layout (8 elems per
                # lane), then transpose back and broadcast across partitions
                dT = ep_pool.tile([128, 4, 2], f32, tag="dT")
                for h in range(2):
                    nc.sync.dma_start(
                        dT[:, :, h], dtmp[h].rearrange("(c p) -> p c", p=128))
                denT = ep_pool.tile([128, 4, 2], f32, tag="denT")
                nc.vector.reciprocal(denT[:], dT[:])
                for h in range(2):
                    nc.sync.dma_start(
                        rden_d[h].rearrange("(c p) -> p c", p=128), denT[:, :, h])
                rdenb0 = ep_pool.tile([HD, NQB], f32, tag="rdenb0")
                nc.sync.dma_start(rdenb0[:], rden_d[0:1, :].partition_broadcast(HD))
                rdenb1 = ep_pool.tile([HD, NQB], f32, tag="rdenb1")
                nc.sync.dma_start(rdenb1[:], rden_d[1:2, :].partition_broadcast(HD))
                # normalize the staged accumulators on the (idle) GpSimd
                sts0 = ep_pool.tile([HD, NQB], bf16, tag="sts0")
                nc.gpsimd.tensor_tensor(sts0[:], st0[:], rdenb0[:], mult)
                sts1 = ep_pool.tile([HD, NQB], bf16, tag="sts1")
                nc.gpsimd.tensor_tensor(sts1[:], st1[:], rdenb1[:], mult)

                def make_proj(cc, sts0=sts0, sts1=sts1, q0=q0):
                    def emit_proj():
                        n0 = q0 + cc * 128
                        yp = ps_s.tile([128, 2 * NQB], f32, tag="s_t",
                                       name=f"yy_{q0}_{cc}")
                        nc.tensor.matmul(
                            yp[:, 0:D], lhsT=sts0[:, cc * 128:(cc + 1) * 128],
                            rhs=wT_sb[:, 0, :], start=True, stop=False)
                        nc.tensor.matmul(
                            yp[:, 0:D], lhsT=sts1[:, cc * 128:(cc + 1) * 128],
                            rhs=wT_sb[:, 1, :], start=False, stop=True)
                        y_sb = ep_pool.tile([128, D], f32, tag="y_sb")
                        nc.vector.tensor_copy(y_sb[:], yp[:, 0:D])
                        nc.sync.dma_start(y[n0:n0 + 128, :], y_sb[:])
                    return emit_proj

                pending_projs = [make_proj(cc) for cc in range(4)]
            for p in pending_projs:
                p()

    nc.compile()
    return nc


def _get_compiled():
    global _compiled
    if _compiled is None:
        _compiled = _build_nc()
    return _compiled


def _prep_core_inputs(x, proj_w):
    """Host-side shard + layout per core: core c -> batch c//4, head pair c%4."""
    import ml_dtypes
    bf16 = ml_dtypes.bfloat16
    ins = []
    for c in range(NCORES):
        b, hp = c // 4, c % 4
        sl = slice(128 * hp, 128 * hp + 128)
        qT = np.ascontiguousarray(x[0, b, :, sl].T).astype(bf16)
        kT = np.ascontiguousarray(x[1, b, :, sl].T).astype(bf16)
        v = x[2, b, :, sl]                       # [N, 128]
        vI = np.ones((128, NCHUNK, 2, HD + 1), np.float32)
        vr = v.reshape(NCHUNK, 128, 2, HD)        # [chunk, p, head, m]
        vI[:, :, :, :HD] = vr.transpose(1, 0, 2, 3)
        wT = np.ascontiguousarray(
            proj_w[:, sl].T.reshape(2, HD, D).transpose(1, 0, 2))  # [HD, 2, D]
        ins.append({"qT": qT, "kT": kT, "vI": vI.astype(bf16),
                    "wT": wT.astype(bf16)})
    return ins


def kernel(x, proj_w, proj_b):
    from concourse.bass_utils import run_bass_kernel_spmd

    x = np.asarray(x, dtype=np.float32)
    proj_w = np.asarray(proj_w, dtype=np.float32)
    proj_b = np.asarray(proj_b, dtype=np.float32)

    nc = _get_compiled()
    in_maps = _prep_core_inputs(x, proj_w)
    res = run_bass_kernel_spmd(nc, in_maps, core_ids=list(range(NCORES)))

    out = np.zeros((B, N, D), np.float32)
    for c in range(NCORES):
        out[c // 4] += res.results[c]["y"]
    out += proj_b
    return out
